# revision 31
# baseline (speedup 1.0000x reference)
"""Trainium2 Bass kernel for nn_Baseline_9904194584728.

Pipeline: embedding gathers + MLP (293->64->64->64->9) + pnerf scan.

Key ideas:
  * Fold W0 into the embedding tables: KW = kmer_embed @ W0[16:272]
    (10648x64, folded on host - a 3 ms GEMM), SW = seq_embed @ W0[:16] + b0
    (20x64, folded on device). Tables stored as bf16 hi|lo pairs packed in
    128-wide rows (256B) so dma_gather's transpose mode lands them directly
    in [feature, batch] layout; a stacked [I64;I64] identity matmul
    reconstitutes hi+lo into fp32 PSUM exactly.
  * The wall clock is dominated by the axon host<->device tunnel
    (~43 MB/s up, ~21 MB/s down, ~50 ms fixed cost per array), so the
    host interface is minimized: ONE uint8 blob per core carrying
      - this core's 1/8 shard of the folded KW table as exact bf16 hi|lo
        pairs (AllGathered across the 8 cores on device),
      - pssm quantized to uint16 (it is uniform in [0,1); pnerf's
        normalizations amplify srf errors ~5000x, so 8 bits is fatal),
      - gather indices as int16 (replicated 8x across partitions
        on device, not on the wire),
      - the small MLP weights in fp32.
    The output is returned as u8 with a per-chunk dynamic scale.
  * pnerf is algebraically an associative prefix product of rigid
    transforms: R_{i+1} = R_i M_i, c_{i+1} = c_i + R_i t_i where
    M_i = [ct^, n^ x ct^, n^], n^ = normalize(e1 x ct), t_i = ct_i.
    The device scan does: pointwise M build -> 24-step within-chunk
    prefix over 128 chunks (batched on partitions) -> hierarchical
    chunk-carry prefix -> batched apply of boundary transforms.
  * Data-parallel over B across the 8 cores (B_s = 32 per core).
"""

import sys
sys.path.insert(0, "/opt/trn_rl_repo")

import numpy as np
import ml_dtypes
from contextlib import ExitStack

import concourse.bass as bass
import concourse.tile as tile
from concourse import bacc, mybir

F32 = mybir.dt.float32
F16 = mybir.dt.float16
BF16 = mybir.dt.bfloat16
I16 = mybir.dt.int16
U8 = mybir.dt.uint8
AL = mybir.AluOpType
AF = mybir.ActivationFunctionType

NCORE = 8
L = 1024
B = 256
BS = B // NCORE            # 32 batch per core
TOK = L * BS               # 32768 tokens per core
NT = TOK // 512            # 64 batch-tiles of 512
NSUP = 8                   # supertiles of 4096 tokens (gather granularity)
NKMER = 10648
KSH = NKMER // NCORE       # 1331 table rows per core
KSHP = 1344                # padded shard rows (512B-aligned collective)
NKP = NCORE * KSHP         # 10752 rows in the gathered table
N3 = 3 * L                 # 3072 chain length
S = 24                     # chunk size (level-1)
C = N3 // S                # 128 chunks
EPS2 = 1e-24

# ---- blob layout (bytes, every section 256-aligned) ----
# pnerf's normalizations amplify srf perturbations ~5000x (absmax), so the
# wire precision floor is ~16 bits/value: pssm ships as u16/65535, and the
# folded kmer table KW = kmer_embed @ W0[16:272] ships as exact bf16 hi|lo
# pairs (folded on host: a 3 ms GEMM that halves the table's wire size).
SZ_KWS = KSHP * 128 * 2        # bf16 [1344, 128] folded-table shard
SZ_PSSM = 84 * 8192 * 2        # uint16 [84, 8192], value = q / 65535
SZ_IDX = 16 * (TOK // 16) * 2  # int16 [16, 2048] (kmer ids)
SZ_SIDX = 16 * (TOK // 16)     # uint8 [16, 2048] (seq ids < 20)
OFF_KWS = 0
OFF_PSSM = OFF_KWS + SZ_KWS
OFF_KIDX = OFF_PSSM + SZ_PSSM
OFF_SIDX = OFF_KIDX + SZ_IDX
OFF_W0P4 = OFF_SIDX + SZ_SIDX          # f32 [21, 64]
OFF_SWET = OFF_W0P4 + 21 * 64 * 4      # f32 [16, 20]
OFF_W0S = OFF_SWET + 16 * 20 * 4       # f32 [16, 64]
OFF_B0 = OFF_W0S + 16 * 64 * 4         # f32 [64]
OFF_WE = OFF_B0 + 256                  # f32 [64, 64]
OFF_W1 = OFF_WE + 64 * 64 * 4          # f32 [64, 9]
OFF_BE = OFF_W1 + 64 * 9 * 4           # f32 [64]
OFF_B1 = OFF_BE + 256                  # f32 [9] (padded to 256)
OFF_IDTF = OFF_B1 + 256                # f32 [384]
OFF_IDK = OFF_IDTF + 384 * 4           # bf16 [128, 64]
NBYTES = ((OFF_IDK + 128 * 64 * 2 + 511) // 512) * 512


# --------------------------------------------------------------------------
# device kernel builder
# --------------------------------------------------------------------------

def _compose_views(t_ap, mode):
    """Return (pcol, arow, outv, col3) view factories for a [128, 384]
    transform tile.

    mode 'mj':  free = m*32 + lane   (m-major; lane = j or ch, 32 lanes)
    mode 'lm':  free = lane*12 + m   (lane-major)
    All views have dims (b, a, lane) with counts (4, 3, 32).
    """
    if mode == 'mj':
        def pcol(cc):
            v = t_ap[:, 3 * cc * 32:(3 * cc + 3) * 32]
            v = v.rearrange("p (a j) -> p a j", a=3)
            return v.unsqueeze(1).broadcast_to([128, 4, 3, 32])

        def arow(cc):
            v = t_ap[:, 0:384].rearrange("p (b three j) -> p b three j",
                                         b=4, three=3)
            v = v[:, :, cc, :]
            return v.unsqueeze(2).broadcast_to([128, 4, 3, 32])

        def outv():
            return t_ap[:, 0:384].rearrange("p (b a j) -> p b a j", b=4, a=3)

        def col3():
            return t_ap[:, 288:384]
    else:  # 'lm'
        def pcol(cc):
            v = t_ap[:, 0:384].rearrange("p (lan m) -> p lan m", lan=32)
            v = v[:, :, 3 * cc:3 * cc + 3]          # [p, lan, a]
            v = v.transpose([0, 2, 1])              # [p, a, lan]
            return v.unsqueeze(1).broadcast_to([128, 4, 3, 32])

        def arow(cc):
            v = t_ap[:, 0:384].rearrange("p (lan b three) -> p lan b three",
                                         lan=32, b=4)
            v = v[:, :, :, cc]                      # [p, lan, b]
            v = v.transpose([0, 2, 1])              # [p, b, lan]
            return v.unsqueeze(2).broadcast_to([128, 4, 3, 32])

        def outv():
            v = t_ap[:, 0:384].rearrange("p (lan b a) -> p lan b a",
                                         lan=32, b=4)
            return v.transpose([0, 2, 3, 1])        # [p, b, a, lan]

        def col3():
            v = t_ap[:, 0:384].rearrange("p (lan m) -> p lan m", lan=32)
            return v[:, :, 9:12]                    # [p, lan, a]
    return pcol, arow, outv, col3


def _emit_compose(nc, dst, P, A, tmpM, tmp2, mode):
    """dst = P o A for transform tiles [128, 384] in the given layout.
    tmpM/tmp2 are scratch [128, 384] tiles (same layout assumed; only
    used through the same view factories)."""
    Pp, _, _, Pc3 = _compose_views(P, mode)
    _, Aa, _, _ = _compose_views(A, mode)
    _, _, Mo, _ = _compose_views(tmpM, mode)
    _, _, To, _ = _compose_views(tmp2, mode)
    Dp, _, Do, Dc3 = _compose_views(dst, mode)
    nc.vector.tensor_tensor(Mo(), Pp(0), Aa(0), AL.mult)
    nc.vector.tensor_tensor(To(), Pp(1), Aa(1), AL.mult)
    nc.vector.tensor_tensor(tmpM[:, 0:384], tmpM[:, 0:384], tmp2[:, 0:384],
                            AL.add)
    nc.vector.tensor_tensor(To(), Pp(2), Aa(2), AL.mult)
    nc.vector.tensor_tensor(dst[:, 0:384], tmpM[:, 0:384], tmp2[:, 0:384],
                            AL.add)
    # translation: dst.t += P.t
    nc.vector.tensor_tensor(Dc3(), Dc3(), Pc3(), AL.add)


def build_nc():
    nc = bacc.Bacc("TRN2", target_bir_lowering=False, debug=False,
                   num_devices=NCORE)

    # ---------------- I/O ----------------
    # output: u8-quantized coordinates (per-partition dynamic scale packed
    # in the last 8 bytes of each row) — halves the down-link bytes
    d_blob = nc.declare_dram_parameter("blob", [NBYTES], U8, isOutput=False)
    o_scan = nc.declare_dram_parameter("o_scan", [128, 2312], U8,
                                       isOutput=True)
    blob = d_blob.ap()

    def sec(off, nbytes, dt, p):
        v = blob[off:off + nbytes].bitcast(dt)
        return v.rearrange("(p n) -> p n", p=p)

    kws = sec(OFF_KWS, SZ_KWS, BF16, KSHP)         # [1344, 128]
    pssm16 = sec(OFF_PSSM, SZ_PSSM, mybir.dt.uint16, 84)   # [84, 8192]
    kidx = sec(OFF_KIDX, SZ_IDX, I16, 16)          # [16, 2048]
    sidx = sec(OFF_SIDX, SZ_SIDX, U8, 16)          # [16, 2048] u8
    w0p4 = sec(OFF_W0P4, 21 * 64 * 4, F32, 21)     # [21, 64]
    swet = sec(OFF_SWET, 16 * 20 * 4, F32, 16)     # [16, 20]
    w0s = sec(OFF_W0S, 16 * 64 * 4, F32, 16)       # [16, 64]
    b0row = sec(OFF_B0, 64 * 4, F32, 1)            # [1, 64]
    we = sec(OFF_WE, 64 * 64 * 4, F32, 64)         # [64, 64]
    w1 = sec(OFF_W1, 64 * 9 * 4, F32, 64)          # [64, 9]
    becol = sec(OFF_BE, 64 * 4, F32, 64)           # [64, 1]
    b1col = sec(OFF_B1, 9 * 4, F32, 9)             # [9, 1]
    idtf = sec(OFF_IDTF, 384 * 4, F32, 1)          # [1, 384]
    idk = sec(OFF_IDK, 128 * 64 * 2, BF16, 128)    # [128, 64]

    # ---------------- internal DRAM ----------------
    kw_shard = nc.dram_tensor("kw_shard", [KSHP, 128], BF16)
    kwp = nc.dram_tensor("kwp", [NKP, 128], BF16)
    swp = nc.dram_tensor("swp", [20, 128], BF16)
    srf_d = nc.dram_tensor("srf_d", [9, TOK], F32)
    d_tc2 = nc.dram_tensor("d_tc2", [128, 384], F32)
    d_g = nc.dram_tensor("d_g", [128, 12], F32)
    d_b2 = nc.dram_tensor("d_b2", [128, 384], F32)

    with ExitStack() as ctx:
        tc = ctx.enter_context(tile.TileContext(nc))

        # persistent pool
        pw = ctx.enter_context(tc.tile_pool(name="pw", bufs=1))
        t_w0p4 = pw.tile([128, 64], F32, tag="w0p4")
        t_we = pw.tile([64, 64], F32, tag="we")
        t_w1 = pw.tile([64, 9], F32, tag="w1")
        t_be = pw.tile([64, 1], F32, tag="be")
        t_b1 = pw.tile([9, 1], F32, tag="b1")
        t_idk = pw.tile([128, 64], BF16, tag="idk")
        t_kidx = pw.tile([128, TOK // 16], I16, tag="kidx")
        t_sidx = pw.tile([128, TOK // 16], I16, tag="sidx")
        t_pssm = pw.tile([128, 8192], F32, tag="pssm")

        for q in range(4):   # quadrant layout for tile_position matmuls
            nc.sync.dma_start(t_w0p4[32 * q:32 * q + 21, :], w0p4)
        nc.sync.dma_start(t_we[:], we)
        nc.sync.dma_start(t_w1[:], w1)
        nc.sync.dma_start(t_be[:], becol)
        nc.sync.dma_start(t_b1[:], b1col)
        nc.sync.dma_start(t_idk[:], idk)
        # indices land on the wire once ([16, n]) and are replicated to the
        # 128-partition layout dma_gather wants here, on device
        nc.sync.dma_start(t_kidx[:],
                          kidx.unsqueeze(0).broadcast_to([8, 16, TOK // 16]))

        # pssm: uint16 [84, 8192] -> f32 [128, 8192] in the 4x32-row quadrant
        # layout phase B's tile_position matmuls want (dequant by 1/65535)
        with ExitStack() as pctx:
            pp = pctx.enter_context(tc.tile_pool(name="pp", bufs=1))
            stag = pp.tile([128, 8192], mybir.dt.uint16, tag="pstag")
            for q in range(4):
                nc.sync.dma_start(stag[32 * q:32 * q + 21, :],
                                  pssm16[21 * q:21 * q + 21, :])
            nc.scalar.activation(t_pssm[:], stag[:], AF.Copy,
                                 scale=1.0 / 65535.0)
            # seq ids arrive as u8; widen to the i16 layout dma_gather wants
            s8 = pp.tile([128, TOK // 16], U8, tag="sidx8")
            nc.sync.dma_start(s8[:],
                              sidx.unsqueeze(0).broadcast_to([8, 16, TOK // 16]))
            nc.scalar.activation(t_sidx[:], s8[:], AF.Copy)

        # ---------------- phase A: tables ----------------
        with ExitStack() as actx:
            a1 = actx.enter_context(tc.tile_pool(name="a1", bufs=1))
            apsum = actx.enter_context(
                tc.tile_pool(name="aps", bufs=2, space="PSUM"))

            # SW table: (20,64) = swet.T @ w0s + b0
            t_swet = a1.tile([16, 20], F32)
            t_w0s = a1.tile([16, 64], F32)
            t_b0r = a1.tile([20, 64], F32)
            nc.sync.dma_start(t_swet[:], swet)
            nc.sync.dma_start(t_w0s[:], w0s)
            nc.sync.dma_start(t_b0r[:], b0row.broadcast_to([20, 64]))
            ps_sw = apsum.tile([20, 64], F32)
            nc.tensor.matmul(ps_sw[:], t_swet[:], t_w0s[:], start=True,
                             stop=True)
            t_swf = a1.tile([20, 64], F32)
            nc.vector.tensor_tensor(t_swf[:], ps_sw[:], t_b0r[:], AL.add)
            t_swpk = a1.tile([20, 128], BF16)
            nc.scalar.activation(t_swpk[:, 0:64], t_swf[:], AF.Copy)
            nc.vector.tensor_tensor(t_swpk[:, 64:128], t_swf[:],
                                    t_swpk[:, 0:64], AL.subtract)
            nc.sync.dma_start(swp[:, :], t_swpk[:])

            # this core's shard of the host-folded KW table -> local DRAM
            nc.sync.dma_start(kw_shard.ap(), kws)

            # assemble the full folded table across the 8 cores
            nc.gpsimd.collective_compute(
                "AllGather", AL.bypass,
                replica_groups=[list(range(NCORE))],
                ins=[kw_shard.ap()], outs=[kwp.ap()])

        # ---------------- phase B: MLP ----------------
        with ExitStack() as bctx:
            gp = bctx.enter_context(tc.tile_pool(name="gp", bufs=2))
            hb = bctx.enter_context(tc.tile_pool(name="hb", bufs=3))
            bps = bctx.enter_context(
                tc.tile_pool(name="bps", bufs=3, space="PSUM"))
            sps = bctx.enter_context(
                tc.tile_pool(name="sps", bufs=2, space="PSUM"))
            sf = bctx.enter_context(tc.tile_pool(name="sf", bufs=2))

            GW = TOK // NSUP                     # 4096 idx per gather
            for sup in range(NSUP):
                kg = gp.tile([128, GW], BF16, tag="kg")
                sg = gp.tile([128, GW], BF16, tag="sg")
                isl = slice(sup * (GW // 16), (sup + 1) * (GW // 16))
                nc.gpsimd.dma_gather(
                    kg[:].rearrange("p (one n) -> p one n", one=1),
                    kwp[:, :], t_kidx[:, isl], num_idxs=GW, num_idxs_reg=GW,
                    elem_size=128, transpose=True, single_packet=False)
                nc.gpsimd.dma_gather(
                    sg[:].rearrange("p (one n) -> p one n", one=1),
                    swp[:, :], t_sidx[:, isl], num_idxs=GW, num_idxs_reg=GW,
                    elem_size=128, transpose=True, single_packet=False)
                srfS = sf.tile([9, GW], F32, tag="srfS")
                for tp in range(NT // NSUP):     # 8 batch-tiles per supertile
                    t = sup * (NT // NSUP) + tp
                    q, r = t % 4, t // 4
                    csl = slice(tp * 512, (tp + 1) * 512)
                    ph0 = bps.tile([64, 512], F32, tag="ph")
                    nc.tensor.matmul(ph0[:], t_idk[:], kg[:, csl],
                                     start=True, stop=False)
                    nc.tensor.matmul(ph0[:], t_idk[:], sg[:, csl],
                                     start=False, stop=False)
                    nc.tensor.matmul(
                        ph0[:], t_w0p4[32 * q:32 * q + 21, :],
                        t_pssm[32 * q:32 * q + 21, 512 * r:512 * r + 512],
                        start=False, stop=True,
                        tile_position=(32 * q, 0))
                    h0 = hb.tile([64, 512], F32, tag="h0")
                    nc.scalar.activation(h0[:], ph0[:], AF.Copy)
                    ph1 = bps.tile([64, 512], F32, tag="ph")
                    nc.tensor.matmul(ph1[:], t_we[:], h0[:], start=True,
                                     stop=True)
                    h1 = hb.tile([64, 512], F32, tag="h1")
                    nc.vector.tensor_scalar(h1[:], ph1[:], t_be[:], 0.0,
                                            AL.add, AL.max)
                    ph2 = bps.tile([64, 512], F32, tag="ph")
                    nc.tensor.matmul(ph2[:], t_we[:], h1[:], start=True,
                                     stop=True)
                    h2 = hb.tile([64, 512], F32, tag="h2")
                    nc.scalar.activation(h2[:], ph2[:], AF.Relu, bias=t_be[:],
                                         scale=1.0)
                    ps3 = sps.tile([9, 512], F32, tag="ps3")
                    nc.tensor.matmul(ps3[:], t_w1[:], h2[:], start=True,
                                     stop=True)
                    nc.vector.tensor_scalar(srfS[:, csl], ps3[:], t_b1[:],
                                            None, AL.add)
                nc.sync.dma_start(srf_d[:, sup * GW:(sup + 1) * GW], srfS[:])

        # ---------------- phase C: scan ----------------
        cp = ctx.enter_context(tc.tile_pool(name="cp", bufs=1))
        ct_all = cp.tile([128, 2304], F32, tag="ct")
        A_all = cp.tile([128, 24 * 384], F32, tag="Aall")
        q_all = cp.tile([128, 2304], F32, tag="qall")
        p_all = cp.tile([128, 2304], F32, tag="pall")
        sq_all = cp.tile([128, 2304], F32, tag="sqall")
        tmp768a = cp.tile([128, 768], F32, tag="t768a")
        tmp768b = cp.tile([128, 768], F32, tag="t768b")
        n2t = cp.tile([128, 768], F32, tag="n2")
        n2ct = cp.tile([128, 768], F32, tag="n2c")
        rnt = cp.tile([128, 768], F32, tag="rn")
        rnct = cp.tile([128, 768], F32, tag="rnc")
        t_idtf = cp.tile([128, 384], F32, tag="idtf")
        nc.sync.dma_start(t_idtf[:], idtf.broadcast_to([128, 384]))

        # C0: permute srf -> ct_all [c, (k*3+x)*32 + j]
        srf_r = srf_d.ap().rearrange("(r x) (c k1 j) -> r x c k1 j",
                                     r=3, x=3, c=128, k1=8)
        ct_r = ct_all[:].rearrange("p (k1 k2 x j) -> p k1 k2 x j",
                                   k1=8, k2=3, x=3)
        for k2 in range(3):
            for x in range(3):
                src = srf_r[k2, x]                       # [c, k1, j]
                nc.sync.dma_start(ct_r[:, :, k2, x, :], src)

        # C1: pointwise transform build
        ctv4 = ct_all[:].rearrange("p (k x j) -> p k x j", k=24, x=3)
        sqv4 = sq_all[:].rearrange("p (k x j) -> p k j x", k=24, x=3)
        Af = A_all[:].rearrange("p (k m j) -> p k m j", k=24, m=12)
        n2v = n2t[:].rearrange("p (k j) -> p k j", k=24)
        n2cv = n2ct[:].rearrange("p (k j) -> p k j", k=24)
        rnv3 = rnt[:].rearrange("p (k j) -> p k j", k=24).unsqueeze(2) \
                     .broadcast_to([128, 24, 3, 32])
        rncv = rnct[:].rearrange("p (k j) -> p k j", k=24)

        def ctx_(x):
            return ctv4[:, :, x, :]

        nc.scalar.activation(sq_all[:], ct_all[:], AF.Square)
        nc.vector.tensor_reduce(n2v.unsqueeze(-1), sqv4, mybir.AxisListType.X,
                                AL.add)
        nc.vector.tensor_reduce(n2cv.unsqueeze(-1), sqv4[:, :, :, 1:3],
                                mybir.AxisListType.X, AL.add)
        nc.vector.tensor_scalar_max(n2t[:], n2t[:], EPS2)
        nc.vector.tensor_scalar_max(n2ct[:], n2ct[:], EPS2)
        nc.scalar.activation(tmp768a[:], n2t[:], AF.Sqrt)
        nc.scalar.activation(tmp768b[:], n2ct[:], AF.Sqrt)
        nc.vector.reciprocal_approx_accurate(rnt[:], tmp768a[:], sq_all[:, 0:768])
        nc.vector.reciprocal_approx_accurate(rnct[:], tmp768b[:],
                                             sq_all[:, 768:1536])
        # A columns: c0 = ct*rn ; t = ct ; c2 = (0, -z*rnc, y*rnc)
        nc.vector.tensor_tensor(Af[:, :, 0:3, :], ctv4, rnv3, AL.mult)
        nc.scalar.activation(Af[:, :, 9:12, :], ctv4, AF.Copy)
        nc.vector.tensor_scalar_mul(Af[:, :, 6, :], ctx_(0), 0.0)
        nc.vector.scalar_tensor_tensor(Af[:, :, 7, :], ctx_(2), -1.0, rncv,
                                       AL.mult, AL.mult)
        nc.vector.tensor_tensor(Af[:, :, 8, :], ctx_(1), rncv, AL.mult)
        # c1 = n^ x c0^
        nc.vector.tensor_tensor(Af[:, :, 3, :], Af[:, :, 7, :],
                                Af[:, :, 2, :], AL.mult)
        nc.vector.tensor_tensor(tmp768a[:].rearrange("p (k j) -> p k j", k=24),
                                Af[:, :, 8, :], Af[:, :, 1, :], AL.mult)
        nc.vector.tensor_tensor(Af[:, :, 3, :], Af[:, :, 3, :],
                                tmp768a[:].rearrange("p (k j) -> p k j", k=24),
                                AL.subtract)
        nc.vector.tensor_tensor(Af[:, :, 4, :], Af[:, :, 8, :],
                                Af[:, :, 0, :], AL.mult)
        nc.vector.scalar_tensor_tensor(Af[:, :, 5, :], Af[:, :, 7, :], -1.0,
                                       Af[:, :, 0, :], AL.mult, AL.mult)

        # C2: level-1 scan (23 steps over k)
        Pa = cp.tile([128, 384], F32, tag="Pa")
        Pb = cp.tile([128, 384], F32, tag="Pb")
        tmpM = cp.tile([128, 384], F32, tag="tmpM")
        tmp2 = cp.tile([128, 384], F32, tag="tmp2")
        nc.scalar.activation(Pa[:], A_all[:, 0:384], AF.Copy)
        nc.scalar.activation(q_all[:, 0:96], A_all[:, 288:384], AF.Copy)
        cur, nxt = Pa, Pb
        for k in range(1, S):
            Ak = A_all[:, k * 384:(k + 1) * 384]
            _emit_compose(nc, nxt, cur, Ak, tmpM, tmp2, 'mj')
            nc.scalar.activation(q_all[:, k * 96:(k + 1) * 96],
                                 nxt[:, 288:384], AF.Copy)
            cur, nxt = nxt, cur
        Pfin = cur

        # C3: level-2 (chunk-carry exclusive prefix)
        # chunk c = 32*cl + ch; level-2 lanes: partition p = j + 32*cl,
        # free lanes ch (32), so all partition slices stay contiguous.
        # repack [c, m*32+j] -> [c, j*12+m] and bounce
        Palt = cp.tile([128, 384], F32, tag="Palt")
        nc.vector.tensor_copy(
            Palt[:].rearrange("p (j m) -> p j m", j=32),
            Pfin[:].rearrange("p (m j) -> p m j", m=12).transpose([0, 2, 1]))
        nc.sync.dma_start(d_tc2[:, :], Palt[:])
        T2 = cp.tile([128, 384], F32, tag="T2")
        tc2r = d_tc2.ap().rearrange("c (j m) -> c j m", j=32)
        for cl in range(4):
            # dst partitions j (block cl) <- rows c = 32*cl + ch
            src = tc2r[32 * cl:32 * cl + 32].transpose([1, 0, 2])  # [j, ch, m]
            nc.sync.dma_start(
                T2[32 * cl:32 * cl + 32, :]
                .rearrange("p (ch m) -> p ch m", ch=32), src)

        # inclusive hierarchical scan over ch (4 blocks x 8) on T2
        chS = cp.tile([128, 384], F32, tag="chS")
        nc.vector.tensor_copy(chS[:], T2[:])

        def lane_views(t_ap, lanes):
            """views for compose on lane-slices of an 'lm' tile; lanes is a
            list/slice spec (lo, n, step) on the 32 lanes."""
            lo, n, step = lanes
            base = t_ap[:, 0:384].rearrange("p (lan m) -> p lan m", lan=32)
            idx = base[:, lo:lo + (n - 1) * step + 1:step, :] if step > 1 \
                else base[:, lo:lo + n, :]
            return idx  # [p, n, 12]

        def compose_lanes(dst_l, P_l, A_l, nl):
            """compose on [p, nl, 12] lane views (dims b,a,lane)."""
            def mk(v):
                pc = v[:, :, 0:9].rearrange("p n (c a) -> p n c a", c=3)

                def pcol(cc):
                    return pc[:, :, cc, :].transpose([0, 2, 1]) \
                        .unsqueeze(1).broadcast_to([128, 4, 3, nl])

                ar = v.rearrange("p n (b three) -> p n b three", b=4)

                def arow(cc):
                    return ar[:, :, :, cc].transpose([0, 2, 1]) \
                        .unsqueeze(2).broadcast_to([128, 4, 3, nl])

                def outv():
                    return v.rearrange("p n (b a) -> p b a n", b=4)

                def col3():
                    return v[:, :, 9:12]
                return pcol, arow, outv, col3

            Pp, _, _, Pc3 = mk(P_l)
            _, Aa, _, _ = mk(A_l)
            tM = lane_views(tmpM, (0, nl, 1))
            t2 = lane_views(tmp2, (0, nl, 1))
            _, _, Mo, _ = mk(tM)
            _, _, To, _ = mk(t2)
            _, _, Do, Dc3 = mk(dst_l)
            nc.vector.tensor_tensor(Mo(), Pp(0), Aa(0), AL.mult)
            nc.vector.tensor_tensor(To(), Pp(1), Aa(1), AL.mult)
            nc.vector.tensor_tensor(Mo(), Mo(), To(), AL.add)
            nc.vector.tensor_tensor(To(), Pp(2), Aa(2), AL.mult)
            nc.vector.tensor_tensor(Do(), Mo(), To(), AL.add)
            nc.vector.tensor_tensor(Dc3(), Dc3(), Pc3(), AL.add)

        for w in range(1, 8):
            # lanes ch = blk*8 + w for blk 0..3
            prev = lane_views(chS, (w - 1, 4, 8))
            curA = lane_views(T2, (w, 4, 8))
            dst = lane_views(chS, (w, 4, 8))
            compose_lanes(dst, prev, curA, 4)

        btot = cp.tile([128, 48], F32, tag="btot")
        btv = btot[:].rearrange("p (n m) -> p n m", n=4)
        nc.vector.tensor_copy(btv[:, 0:1, :], lane_views(chS, (7, 1, 1)))
        for blk in range(1, 4):
            compose_lanes(btv[:, blk:blk + 1, :], btv[:, blk - 1:blk, :],
                          lane_views(chS, (blk * 8 + 7, 1, 1)), 1)

        Pchi = cp.tile([128, 384], F32, tag="Pchi")
        nc.vector.tensor_copy(Pchi[:, 0:96], chS[:, 0:96])
        # blocks 1..3: compose btot[blk-1] (bcast over w) with chS lanes
        for blk in range(1, 4):
            bview = btv[:, blk - 1:blk, :].broadcast_to([128, 8, 12])
            compose_lanes(lane_views(Pchi, (blk * 8, 8, 1)), bview,
                          lane_views(chS, (blk * 8, 8, 1)), 8)

        Pche = cp.tile([128, 384], F32, tag="Pche")
        nc.vector.tensor_copy(Pche[:, 0:12], t_idtf[:, 0:12])
        nc.vector.tensor_copy(Pche[:, 12:384], Pchi[:, 0:372])

        # cross-block (cl) exclusive prefix of block totals via DRAM bounce
        nc.sync.dma_start(d_g[:, :], Pchi[:, 372:384])
        G4 = cp.tile([128, 48], F32, tag="G4")
        for clp in range(4):
            src = d_g.ap()[32 * clp:32 * clp + 32, :]
            src = src.unsqueeze(0).broadcast_to([4, 32, 12])
            nc.sync.dma_start(G4[:, clp * 12:(clp + 1) * 12], src)
        g4v = G4[:].rearrange("p (n m) -> p n m", n=4)
        P01t = cp.tile([128, 12], F32, tag="P01t")
        P012t = cp.tile([128, 12], F32, tag="P012t")
        compose_lanes(P01t[:].unsqueeze(1), g4v[:, 0:1, :], g4v[:, 1:2, :], 1)
        compose_lanes(P012t[:].unsqueeze(1), P01t[:].unsqueeze(1),
                      g4v[:, 2:3, :], 1)
        Pexcl = cp.tile([128, 12], F32, tag="Pexcl")
        nc.vector.tensor_copy(Pexcl[0:32, :], t_idtf[0:32, 0:12])
        nc.vector.tensor_copy(Pexcl[32:64, :], G4[32:64, 0:12])
        nc.vector.tensor_copy(Pexcl[64:96, :], P01t[64:96, :])
        nc.vector.tensor_copy(Pexcl[96:128, :], P012t[96:128, :])

        # B_chunk (in level-2 lane layout) = Pexcl o S_excl
        Bcj = cp.tile([128, 384], F32, tag="Bcj")
        compose_lanes(lane_views(Bcj, (0, 32, 1)),
                      Pexcl[:].unsqueeze(1).broadcast_to([128, 32, 12]),
                      lane_views(Pche, (0, 32, 1)), 32)
        nc.sync.dma_start(d_b2[:, :], Bcj[:])
        Bch = cp.tile([128, 384], F32, tag="Bch")
        b2r = d_b2.ap().rearrange("p (ch m) -> p ch m", ch=32)
        for cl in range(4):
            src = b2r[32 * cl:32 * cl + 32].transpose([1, 0, 2])  # [ch, j, m]
            nc.sync.dma_start(
                Bch[32 * cl:32 * cl + 32, :]
                .rearrange("p (j m) -> p j m", j=32), src)

        # C4: apply  p = B.t + B.R @ q
        qv = q_all[:].rearrange("p (k x j) -> p k x j", k=24, x=3)
        Bv = Bch[:].rearrange("p (j m) -> p j m", j=32)
        pv = p_all[:].rearrange("p (k a j) -> p k a j", k=24, a=3)
        tA = sq_all[:]  # reuse as scratch [128, 2304]
        tAv = tA.rearrange("p (k a j) -> p k a j", k=24, a=3)
        tB = ct_all[:]  # reuse as scratch
        tBv = tB.rearrange("p (k a j) -> p k a j", k=24, a=3)

        def qx(cc):
            return qv[:, :, cc, :].unsqueeze(2).broadcast_to([128, 24, 3, 32])

        def bcol(cc):
            v = Bv[:, :, 3 * cc:3 * cc + 3].transpose([0, 2, 1])  # [p,a,j]
            return v.unsqueeze(1).broadcast_to([128, 24, 3, 32])

        nc.vector.tensor_tensor(tAv, qx(0), bcol(0), AL.mult)
        nc.vector.tensor_tensor(tBv, qx(1), bcol(1), AL.mult)
        nc.vector.tensor_tensor(tAv, tAv, tBv, AL.add)
        nc.vector.tensor_tensor(tBv, qx(2), bcol(2), AL.mult)
        nc.vector.tensor_tensor(tAv, tAv, tBv, AL.add)
        nc.vector.tensor_tensor(pv, tAv, bcol(3), AL.add)
        # quantize the coordinates to u8 with a per-partition (per-chunk)
        # dynamic scale; ship the scale in bytes 2304:2308 of each row
        nc.scalar.activation(A_all[:, 0:2304], p_all[:], AF.Abs)
        mrow = cp.tile([128, 1], F32, tag="mrow")
        nc.vector.tensor_reduce(mrow[:].unsqueeze(1),
                                A_all[:, 0:2304].unsqueeze(1),
                                mybir.AxisListType.X, AL.max)
        nc.vector.tensor_scalar_max(mrow[:], mrow[:], 1e-30)
        rrow = cp.tile([128, 1], F32, tag="rrow")
        nc.vector.reciprocal(rrow[:], mrow[:])
        nc.vector.tensor_scalar_mul(rrow[:], rrow[:], 127.0)
        qout = cp.tile([128, 2304], U8, tag="qout")
        nc.scalar.activation(qout[:], p_all[:], AF.Copy, scale=rrow[:],
                             bias=128.0)
        nc.sync.dma_start(o_scan[:, 0:2304], qout[:])
        nc.sync.dma_start(o_scan[:, 2304:2308], rrow[:].bitcast(U8))

    nc.compile()
    return nc


# --------------------------------------------------------------------------
# host wrapper: cached jit over the bass_exec custom call
# --------------------------------------------------------------------------

_RT = {}


def _get_rt():
    if _RT:
        return _RT
    import jax
    import jax.numpy as jnp
    from jax.sharding import Mesh, PartitionSpec, NamedSharding
    from jax.experimental.shard_map import shard_map
    from concourse.bass2jax import (_bass_exec_p, install_neuronx_cc_hook,
                                    partition_id_tensor)

    nc = build_nc()
    install_neuronx_cc_hook()

    partition_name = (nc.partition_id_tensor.name
                      if nc.partition_id_tensor else None)
    in_names, out_names, out_avals = [], [], []
    for alloc in nc.m.functions[0].allocations:
        if not isinstance(alloc, mybir.MemoryLocationSet):
            continue
        name = alloc.memorylocations[0].name
        if alloc.kind == "ExternalInput":
            if name != partition_name:
                in_names.append(name)
        elif alloc.kind == "ExternalOutput":
            out_names.append(name)
            out_avals.append(jax.core.ShapedArray(
                tuple(alloc.tensor_shape), mybir.dt.np(alloc.dtype)))
    assert in_names == ["blob"] and out_names == ["o_scan"], \
        (in_names, out_names)
    all_in_names = in_names + out_names
    if partition_name is not None:
        all_in_names.append(partition_name)

    def _body(blob, zeros):
        operands = [blob, zeros]
        if partition_name is not None:
            operands.append(partition_id_tensor())
        outs = _bass_exec_p.bind(
            *operands, out_avals=tuple(out_avals),
            in_names=tuple(all_in_names), out_names=tuple(out_names),
            lowering_input_output_aliases=(),
            sim_require_finite=True, sim_require_nnan=True, nc=nc)
        return outs[0]

    devices = jax.devices()[:NCORE]
    mesh = Mesh(np.asarray(devices), ("core",))
    sh = NamedSharding(mesh, PartitionSpec("core"))
    sharded = jax.jit(
        shard_map(_body, mesh=mesh,
                  in_specs=(PartitionSpec("core"), PartitionSpec("core")),
                  out_specs=PartitionSpec("core"), check_rep=False),
        donate_argnums=(1,), keep_unused=True)
    oshape = tuple(out_avals[0].shape)
    zeros_fn = jax.jit(
        lambda: jnp.zeros((NCORE * oshape[0],) + oshape[1:],
                          out_avals[0].dtype),
        out_shardings=sh)

    _RT.update(nc=nc, sharded=sharded, zeros_fn=zeros_fn, oshape=oshape)
    return _RT


def make_blobs(inputs):
    """Pack the full inputs into one uint8 blob per core."""
    seq = np.asarray(inputs["seq"])
    kmer = np.asarray(inputs["kmer"])
    pssm = np.asarray(inputs["pssm"], dtype=np.float32)
    seq_embed = np.asarray(inputs["seq_embed"], dtype=np.float32)
    kmer_embed = np.asarray(inputs["kmer_embed"], dtype=np.float32)
    W0 = np.asarray(inputs["W0"], dtype=np.float32)
    b0 = np.asarray(inputs["b0"], dtype=np.float32)
    We = np.asarray(inputs["We"], dtype=np.float32)
    be = np.asarray(inputs["be"], dtype=np.float32)
    W1 = np.asarray(inputs["W1"], dtype=np.float32)
    b1 = np.asarray(inputs["b1"], dtype=np.float32)

    buf = _RT.get("blob_buf")
    if buf is None:
        buf = _RT["blob_buf"] = np.zeros((NCORE, NBYTES), np.uint8)
    blobs = buf

    def put(off, arr):
        raw = arr.reshape(-1).view(np.uint8)
        blobs[:, off:off + raw.size] = raw[None, :]

    # shared sections (identical on every core)
    put(OFF_W0P4, np.ascontiguousarray(W0[272:293]))
    put(OFF_SWET, np.ascontiguousarray(seq_embed.T))
    put(OFF_W0S, np.ascontiguousarray(W0[:16]))
    put(OFF_B0, b0)
    put(OFF_WE, We)
    put(OFF_W1, np.ascontiguousarray(W1))
    put(OFF_BE, be)
    put(OFF_B1, b1)
    id12 = np.array([1, 0, 0, 0, 1, 0, 0, 0, 1, 0, 0, 0], np.float32)
    put(OFF_IDTF, np.tile(id12, 32))
    put(OFF_IDK, np.tile(np.eye(64, dtype=ml_dtypes.bfloat16), (2, 1)))

    # per-core sections (vectorized across cores)
    # fold W0 into the kmer table on host (3 ms GEMM) and ship exact
    # bf16 hi|lo pairs: half the wire bytes of raw 16-bit embeddings
    KW = kmer_embed.astype(np.float32) @ W0[16:272]           # (10648, 64)
    kw_hi = KW.astype(ml_dtypes.bfloat16)
    kw_lo = (KW - kw_hi.astype(np.float32)).astype(ml_dtypes.bfloat16)
    kws = np.zeros((NCORE, KSHP, 128), ml_dtypes.bfloat16)
    kws[:, :KSH, 0:64] = kw_hi.reshape(NCORE, KSH, 64)
    kws[:, :KSH, 64:128] = kw_lo.reshape(NCORE, KSH, 64)

    # pssm is in [0, 1): p*65535 + 0.5 stays below 65536, so a plain
    # truncating cast implements round-to-nearest without a clip pass
    # (preallocated scratch keeps the 22 MB quantize pass allocation-free)
    scr = _RT.get("pssm_scr")
    if scr is None:
        scr = _RT["pssm_scr"] = (np.empty(pssm.shape, np.float32),
                                 np.empty(pssm.shape, np.uint16))
    tmp, pssm_q = scr
    np.multiply(pssm, np.float32(65535.0), out=tmp)
    np.add(tmp, np.float32(0.5), out=tmp)
    pssm_q[...] = tmp
    # token g = l*32 + j decomposes as (r, q, l3, j): l = r*64 + q*16 + l3,
    # pack[c, 21q+f, 512r + 32*l3 + j] = pssm_q[l, 32c+j, f]
    X = pssm_q.reshape(16, 4, 16, NCORE, BS, 21)
    ppack = np.ascontiguousarray(X.transpose(3, 1, 5, 0, 2, 4)) \
        .reshape(NCORE, 84 * 8192)

    # remap kmer ids into the padded-shard row space of the gathered table;
    # wrap16: idx g = l*32+j lands at [j%16, 2l + j//16]
    kmap = ((kmer // KSH) * KSHP + kmer % KSH).astype(np.int16)

    def wrap16_all(idx):
        v = idx.reshape(L, NCORE, 2, 16)
        return np.ascontiguousarray(v.transpose(1, 3, 0, 2)) \
            .reshape(NCORE, 16 * (TOK // 16))

    kw = wrap16_all(kmap)
    sw = wrap16_all(seq.astype(np.uint8))

    blobs[:, OFF_KWS:OFF_KWS + SZ_KWS] = \
        kws.reshape(NCORE, -1).view(np.uint8)
    blobs[:, OFF_PSSM:OFF_PSSM + SZ_PSSM] = ppack.view(np.uint8)
    blobs[:, OFF_KIDX:OFF_KIDX + SZ_IDX] = kw.view(np.uint8)
    blobs[:, OFF_SIDX:OFF_SIDX + SZ_SIDX] = sw
    return blobs


def unpack_output(flat):
    """[NCORE*128, 2312] u8 (2304 q-bytes + f32 scale) -> (N3, B, 3) f32."""
    raw = np.asarray(flat).reshape(NCORE, 128, 2312)
    r = np.ascontiguousarray(raw[:, :, 2304:2308]).view(np.float32)
    dec = (raw[:, :, 0:2304].astype(np.float32) - np.float32(128.0)) / r
    arr = dec.reshape(NCORE, 128, 24, 3, 32)
    out = np.empty((N3, B, 3), np.float32)
    for c in range(NCORE):
        out[:, c * BS:(c + 1) * BS, :] = \
            arr[c].transpose(0, 1, 3, 2).reshape(N3, BS, 3)
    return out


def kernel(**inputs):
    rt = _get_rt()
    blobs = make_blobs(inputs)
    # the donated output buffer's contents are never read (the kernel
    # writes every element), so recycle the previous call's output array
    # instead of paying a zeros-dispatch round trip each call
    donate = rt.pop("recycle", None)
    if donate is None:
        donate = rt["zeros_fn"]()
    out = rt["sharded"](blobs.reshape(NCORE * NBYTES), donate)
    res = unpack_output(out)
    rt["recycle"] = out
    return res
